# revision 10
# baseline (speedup 1.0000x reference)
"""Trainium2 Bass kernel for nn_ContinuousSheafTransport (GNN message-passing ODE).

Self-contained: takes FULL inputs, shards across 8 NeuronCores internally,
returns FULL outputs (h_final [N,D], disagreements [T,E]).

Sharding: nodes split 8 ways; each edge lives on the core owning its
destination row; edges sorted by local row so segment_sum is core-local.
Per Euler step each core builds node-level tables hA=h@W1[:D], hB=h@W1[D:2D]
(hB all-gathered); per-edge work reduces to two 256B dma_gathers, a relu, and
a one-hot segment-matmul:
    z_e = relu(hA[row_e] + hB[col_e] + ea_e@W1c + b1)
    x   = h@Wd1h + segsum(z)@(W2@Wd1a) + (deg*b2)@Wd1a + bd1
    h  += 0.5*tanh(silu(LN(x))@Wd2 + bd2)
The node pipeline runs transposed (features on partitions) so LN gamma/beta,
bd2 and tanh are per-partition scalar-engine ops.
"""
import numpy as np
import ml_dtypes

T_STEPS = 2
LAST_EXEC_NS = None
LAST_RES = None
LAST_RUN_WALL = None
STEP = 0.5
EPS = 1e-5
N_CORES = 8
CH = 1024        # gather chunk size in edges (dma_gather crashes above ~1024 idxs)


# --------------------------------------------------------------------------- #
# host-side preprocessing
# --------------------------------------------------------------------------- #

def _wrap_idx(idx):
    """dma_gather int16 idx layout [128, n/16]: element i at [i%16, i//16]."""
    n = len(idx)
    assert n % 16 == 0
    blk = idx.reshape(n // 16, 16).T.astype(np.int16)
    return np.tile(blk, (8, 1))


def _preprocess(h_seq, edge_attr, edge_index, split):
    T, N, D = h_seq.shape
    NC = N_CORES
    assert N % NC == 0
    Nc = N // NC
    NW = (Nc + 127) // 128

    pp = {"T": T, "N": N, "D": D, "E": edge_index.shape[2], "Nc": Nc, "NW": NW,
          "split": split}
    per_t = []
    for t in range(T):
        row = np.asarray(edge_index[t, 0], np.int64)
        col = np.asarray(edge_index[t, 1], np.int64)
        ea = np.asarray(edge_attr[t], np.float32)

        cores, nlo_max, nhi_max = [], 0, 0
        for k in range(NC):
            eids = np.nonzero((row >= k * Nc) & (row < (k + 1) * Nc))[0]
            lr = row[eids] - k * Nc
            is_lo = col[eids] < split
            lo = eids[is_lo][np.argsort(lr[is_lo], kind="stable")]
            hi = eids[~is_lo][np.argsort(lr[~is_lo], kind="stable")]
            cores.append((lo, hi))
            nlo_max, nhi_max = max(nlo_max, len(lo)), max(nhi_max, len(hi))
        NLOP = ((nlo_max + 127) // 128) * 128
        NHIP = ((nhi_max + 127) // 128) * 128
        NEP = NLOP + NHIP
        NTIL = NEP // 128

        eaT = np.zeros((NC, 17, NEP), ml_dtypes.bfloat16)
        col_lo = np.zeros((NC, max(NLOP, 16)), np.int64)
        col_hi = np.zeros((NC, max(NHIP, 16)), np.int64)
        row_l = np.zeros((NC, NEP), np.int64)
        lvalid = np.zeros((NC, NEP), bool)
        eid_pos = []
        for k in range(NC):
            lo, hi = cores[k]
            pos = np.concatenate([np.arange(len(lo)), NLOP + np.arange(len(hi))])
            ids = np.concatenate([lo, hi])
            eid_pos.append((ids, pos))
            eaT[k, :16, pos] = ea[ids].astype(ml_dtypes.bfloat16)
            eaT[k, 16, pos] = np.float32(1.0)
            col_lo[k, :len(lo)] = col[lo]
            col_hi[k, :len(hi)] = col[hi] - split
            row_l[k, pos] = row[ids] - k * Nc
            lvalid[k, pos] = True

        # union-over-cores (tile, window) schedule, per group
        def build_pairs(j0, j1, pbase):
            pair_set = {}
            for k in range(NC):
                for j in range(j0, j1):
                    sl = slice(j * 128, (j + 1) * 128)
                    v = lvalid[k, sl]
                    if not v.any():
                        continue
                    for w in np.unique(row_l[k, sl][v] // 128):
                        pair_set.setdefault(j, set()).add(int(w))
            pairs = []
            for j in range(j0, j1):
                for w in sorted(pair_set.get(j, ())):
                    pairs.append((j, int(w), pbase + len(pairs)))
            wf, wl = {}, {}
            for (j, w, pi) in pairs:
                wf.setdefault(w, pi)
                wl[w] = pi
            return pairs, wf, wl

        pairs_lo, wf_lo, wl_lo = build_pairs(0, NLOP // 128, 0)
        pairs_hi, wf_hi, wl_hi = build_pairs(NLOP // 128, NTIL, len(pairs_lo))
        NPAIR = max(len(pairs_lo) + len(pairs_hi), 1)
        lrowW = np.full((NC, 128, NPAIR), -1.0, np.float32)
        for (j, w, pi) in pairs_lo + pairs_hi:
            sl = slice(j * 128, (j + 1) * 128)
            for k in range(NC):
                lv = row_l[k, sl] - 128 * w
                ok = lvalid[k, sl] & (lv >= 0) & (lv < 128)
                lrowW[k, ok, pi] = lv[ok]

        deg = np.zeros((NC, Nc), np.float32)
        for k in range(NC):
            ids, _ = eid_pos[k]
            np.add.at(deg[k], row[ids] - k * Nc, 1.0)

        per_t.append({
            "NLOP": NLOP, "NHIP": NHIP, "NEP": NEP, "NTIL": NTIL,
            "pairs_lo": pairs_lo, "pairs_hi": pairs_hi,
            "wf_lo": wf_lo, "wl_lo": wl_lo, "wf_hi": wf_hi, "wl_hi": wl_hi,
            "NPAIR": NPAIR,
            "eaT": eaT,
            "idx_lo": np.stack([_wrap_idx(col_lo[k]) for k in range(NC)]),
            "idx_hi": np.stack([_wrap_idx(col_hi[k]) for k in range(NC)]),
            "idx_row": np.stack([_wrap_idx(row_l[k]) for k in range(NC)]),
            "lrowW": lrowW,
            "deg": deg, "eid_pos": eid_pos,
        })
    pp["per_t"] = per_t
    pp["hTseq"] = np.ascontiguousarray(
        h_seq.reshape(T, NC, Nc, D).transpose(1, 0, 3, 2)).astype(np.float32)
    return pp


# --------------------------------------------------------------------------- #
# device program
# --------------------------------------------------------------------------- #

def _build_program(pp):
    import concourse.bacc as bacc
    import concourse.mybir as mybir
    import concourse.tile as tile
    from contextlib import ExitStack
    from concourse.dve_spec import Spec, Src0, Src1, relu as drelu, sq as dsq, \
        lower as dve_lower
    from concourse.dve_uop import DveOpSpec
    from concourse import dve_ops as dvo

    f32, bf16, i16, i32 = (mybir.dt.float32, mybir.dt.bfloat16,
                           mybir.dt.int16, mybir.dt.int32)
    AF = mybir.ActivationFunctionType
    ALU = mybir.AluOpType

    def make_op(name, spec):
        for o in dvo.OPS:
            if o.name == name:
                return o
        shas = {}
        for ver in ("v3", "v4"):
            tmp = DveOpSpec(name=name, opcode=0, uops=dve_lower(spec, ver=ver),
                            rd1_en=True)
            shas[ver] = tmp.sha(ver)
        op = dvo.DveOp(name, spec, subdim=False, uops_sha=shas)
        dvo.OPS.append(op)
        dvo.CUSTOM_DVE_SPECS[name] = spec
        dvo._SUB_OPCODE_FOR_NAME[name] = dvo._CUSTOM_DVE_ROW_BASE + len(dvo.OPS) - 1
        assert dvo._SUB_OPCODE_FOR_NAME[name] < 0x20
        return op

    RELU_ADD = make_op("GNN_RELU_ADD", Spec(
        body=drelu(Src0 + Src1),
        reference=lambda in0, in1, s0, s1, imm2:
            np.maximum(np.nan_to_num(in0.astype(np.float32) + in1,
                                     nan=0.0, posinf=np.inf, neginf=-np.inf), 0)))
    SQDIFF = make_op("GNN_SQDIFF", Spec(
        body=dsq(Src0 - Src1),
        reference=lambda in0, in1, s0, s1, imm2:
            np.square(in0.astype(np.float32) - in1)))

    T, Nc, NW = pp["T"], pp["Nc"], pp["NW"]
    N, split = pp["N"], pp["split"]
    NcP = NW * 128

    nc = bacc.Bacc("TRN2", target_bir_lowering=False, debug=False,
                   num_devices=N_CORES)

    dmi = {}

    def din(name, shape, dt):
        dmi[name] = nc.dram_tensor(name, list(shape), dt, kind="ExternalInput")
        return dmi[name]

    hTseq_d = din("hTseq", [T, 64, Nc], f32)
    WX_d = din("WX", [66, 64], f32)
    W1ab_d = din("W1ab", [64, 128], f32)
    W2d_d = din("W2d", [64, 64], f32)
    W1cb_d = din("W1cb", [17, 64], bf16)
    Wd2_d = din("Wd2", [64, 64], bf16)
    lng_d = din("lng", [64, 1], f32)
    lnb_d = din("lnb", [64, 1], f32)
    bd2_d = din("bd2", [64, 1], f32)
    deg_d = din("degT", [T, 1, Nc], f32)
    for t in range(T):
        s = pp["per_t"][t]
        din(f"eaT{t}", [17, s["NEP"]], bf16)
        din(f"idxlo{t}", [128, max(s["NLOP"], 16) // 16], i16)
        din(f"idxhi{t}", [128, max(s["NHIP"], 16) // 16], i16)
        din(f"idxrow{t}", [128, s["NEP"] // 16], i16)
        din(f"lrowW{t}", [128, s["NPAIR"]], f32)

    hT_out = nc.dram_tensor("hT_out", [64, Nc], f32, kind="ExternalOutput")
    dis_out = {t: nc.dram_tensor(f"dis{t}", [128, pp["per_t"][t]["NTIL"]], f32,
                                 kind="ExternalOutput") for t in range(T)}

    hA_loc2 = [nc.dram_tensor(f"hA_loc{p}", [NcP, 64], f32) for p in range(2)]
    tab_in2 = [nc.dram_tensor(f"tab_in{p}", [Nc, 64], f32) for p in range(2)]
    tab_full2 = [nc.dram_tensor(f"tab_full{p}", [N, 64], f32, addr_space="Shared")
                 for p in range(2)]

    with tile.TileContext(nc) as tc, ExitStack() as ctx:
        sb = ctx.enter_context(tc.tile_pool(name="sb", bufs=1))
        sbr = ctx.enter_context(tc.tile_pool(name="sbr", bufs=3))
        zp = ctx.enter_context(tc.tile_pool(name="zp", bufs=2, space="PSUM"))
        gp = ctx.enter_context(tc.tile_pool(name="gp", bufs=2, space="PSUM"))
        xp = ctx.enter_context(tc.tile_pool(name="xp", bufs=2, space="PSUM"))
        tp = ctx.enter_context(tc.tile_pool(name="tp", bufs=2, space="PSUM"))

        # ---------------- residents ----------------
        hTx = sb.tile([66, NcP], f32)
        nc.vector.memset(hTx[:], 0.0)
        WX = sb.tile([66, 64], f32); nc.sync.dma_start(out=WX[:], in_=WX_d[:])
        W1ab = sb.tile([64, 128], f32); nc.sync.dma_start(out=W1ab[:], in_=W1ab_d[:])
        W2d = sb.tile([64, 64], f32); nc.sync.dma_start(out=W2d[:], in_=W2d_d[:])
        W1cb = sb.tile([17, 64], bf16); nc.sync.dma_start(out=W1cb[:], in_=W1cb_d[:])
        Wd2 = sb.tile([64, 64], bf16); nc.sync.dma_start(out=Wd2[:], in_=Wd2_d[:])
        lng = sb.tile([64, 1], f32); nc.sync.dma_start(out=lng[:], in_=lng_d[:])
        lnb = sb.tile([64, 1], f32); nc.sync.dma_start(out=lnb[:], in_=lnb_d[:])
        bd2 = sb.tile([64, 1], f32); nc.sync.dma_start(out=bd2[:], in_=bd2_d[:])

        nc.vector.memset(hTx[64:65, :], 1.0)      # ones row

        iota_i = sb.tile([128, 128], i32)
        nc.gpsimd.iota(iota_i[:], pattern=[[1, 128]], base=0, channel_multiplier=0)
        iotaF = sb.tile([128, 128], f32)
        nc.vector.tensor_copy(out=iotaF[:], in_=iota_i[:])
        iotaP_i = sb.tile([128, 1], i32)
        nc.gpsimd.iota(iotaP_i[:], pattern=[[0, 1]], base=0, channel_multiplier=1)
        iotaP = sb.tile([128, 1], f32)
        nc.vector.tensor_copy(out=iotaP[:], in_=iotaP_i[:])
        identB = sb.tile([128, 128], bf16)
        nc.vector.tensor_scalar(out=identB[:], in0=iotaF[:], scalar1=iotaP[:],
                                scalar2=None, op0=ALU.is_equal)
        identF = sb.tile([128, 128], f32)
        nc.vector.tensor_copy(out=identF[:], in_=identB[:])

        # h init: h = 2*h_seq[0]
        nc.sync.dma_start(out=hTx[0:64, 0:Nc], in_=hTseq_d[0])
        nc.vector.tensor_scalar_mul(out=hTx[0:64, 0:Nc], in0=hTx[0:64, 0:Nc],
                                    scalar1=2.0)

        gT = sb.tile([64, NcP], f32)
        fT = sb.tile([64, NcP], bf16)
        x_sb = sb.tile([128, NW, 64], f32)
        stat_mu = sb.tile([128, NW], f32)
        stat_sq = sb.tile([128, NW], f32)
        stat_rv = sb.tile([128, NW], f32)
        stat_b = sb.tile([128, NW], f32)

        # ---------------- helpers ----------------
        def node_tables(write_h, par):
            hA_loc, tab_in = hA_loc2[par], tab_in2[par]
            for m in range(NW):
                lo = min(128, Nc - m * 128)
                if lo <= 0:
                    continue
                ps = tp.tile([128, 128], f32, tag="tp")
                st = sbr.tile([128, 128], f32, tag="tabs")
                if write_h:
                    nc.tensor.transpose(ps[:, 0:64],
                                        hTx[0:64, m * 128:(m + 1) * 128],
                                        identF[0:64, 0:64])
                    nc.scalar.activation(out=st[:, 0:64], in_=ps[:, 0:64],
                                         func=AF.Copy)
                    nc.sync.dma_start(out=tab_in[m * 128:m * 128 + lo, :],
                                      in_=st[0:lo, 0:64])
                    nc.sync.dma_start(out=hA_loc[m * 128:m * 128 + lo, :],
                                      in_=st[0:lo, 0:64])
                else:
                    nc.tensor.matmul(ps[:], lhsT=hTx[0:64, m * 128:(m + 1) * 128],
                                     rhs=W1ab[:], start=True, stop=True)
                    nc.scalar.activation(out=st[:], in_=ps[:], func=AF.Copy)
                    nc.sync.dma_start(out=hA_loc[m * 128:m * 128 + lo, :],
                                      in_=st[0:lo, 0:64])
                    nc.sync.dma_start(out=tab_in[m * 128:m * 128 + lo, :],
                                      in_=st[0:lo, 64:128])

        def allgather_tab(par):
            nc.gpsimd.collective_compute(
                "AllGather", ALU.bypass,
                replica_groups=[list(range(N_CORES))],
                ins=[tab_in2[par][:]], outs=[tab_full2[par][:]])

        def edge_pass(t, s, eaT_d, idxlo, idxhi, idxrow, lrowW, compute_z, par):
            hA_loc, tab_full = hA_loc2[par], tab_full2[par]
            NLOP, NHIP, NEP, NTIL = s["NLOP"], s["NHIP"], s["NEP"], s["NTIL"]
            groups = []
            if NLOP:
                groups.append((0, NLOP, idxlo, tab_full[0:min(split, N), :],
                               s["pairs_lo"], s["wf_lo"], s["wl_lo"], "lo"))
            if NHIP:
                groups.append((NLOP, NHIP, idxhi, tab_full[split:N, :],
                               s["pairs_hi"], s["wf_hi"], s["wl_hi"], "hi"))

            if compute_z:
                nc.vector.memset(gT[:], 0.0)
            dis_sb = None
            if not compute_z:
                dis_sb = sb.tile([128, NTIL], f32, tag="dis_sb")

            for (g0, gcnt, idx_t, src, pairs, wf, wl, gname) in groups:
                pair_of = {}
                for (j, w, pi) in pairs:
                    pair_of.setdefault(j, []).append((pi, w))
                win_ps = {}
                nch = (gcnt + CH - 1) // CH
                LOOK = 3
                rg_bufs = {}

                def emit_rowgather(c):
                    e0 = g0 + c * CH
                    ecnt = min(CH, g0 + gcnt - e0)
                    rgb = sbr.tile([128, CH // 128, 64], f32, tag="rg",
                                   name=f"rg_{c}", bufs=LOOK + 2)
                    nc.gpsimd.dma_gather(
                        out_ap=rgb[:, 0:ecnt // 128, :], in_ap=hA_loc[:, :],
                        idxs_ap=idxrow[:, e0 // 16:(e0 + ecnt) // 16],
                        num_idxs=ecnt, num_idxs_reg=ecnt, elem_size=64)
                    rg_bufs[c] = rgb

                for c in range(min(LOOK, nch)):
                    emit_rowgather(c)
                for c in range(nch):
                    e0 = g0 + c * CH
                    ecnt = min(CH, g0 + gcnt - e0)
                    ntc = ecnt // 128
                    if c + LOOK < nch:
                        emit_rowgather(c + LOOK)
                    rg = rg_bufs.pop(c)
                    cg = sbr.tile([128, CH // 128, 64], f32, tag="cg")
                    nc.gpsimd.dma_gather(
                        out_ap=cg[:, 0:ntc, :], in_ap=src,
                        idxs_ap=idx_t[:, (e0 - g0) // 16:(e0 - g0 + ecnt) // 16],
                        num_idxs=ecnt, num_idxs_reg=ecnt, elem_size=64)

                    if not compute_z:
                        d2 = sbr.tile([128, CH // 128, 64], f32, tag="d2")
                        nc.vector._custom_dve(SQDIFF, out=d2[:, 0:ntc, :],
                                              in0=rg[:, 0:ntc, :],
                                              in1=cg[:, 0:ntc, :])
                        nc.vector.tensor_reduce(
                            out=dis_sb[:, e0 // 128:e0 // 128 + ntc],
                            in_=d2[:, 0:ntc, :], axis=mybir.AxisListType.X,
                            op=ALU.add)
                        continue

                    eat_c = sbr.tile([17, CH], bf16, tag="eat_c")
                    nc.sync.dma_start(out=eat_c[:, 0:ecnt],
                                      in_=eaT_d[:, e0:e0 + ecnt])
                    rc = sbr.tile([128, CH // 128, 64], bf16, tag="rc")
                    nc.vector.tensor_tensor(out=rc[:, 0:ntc, :],
                                            in0=rg[:, 0:ntc, :],
                                            in1=cg[:, 0:ntc, :], op=ALU.add)
                    zt = sbr.tile([128, CH // 128, 64], bf16, tag="zt")
                    for b0 in range(0, ntc, 8):
                        nb = min(8, ntc - b0)
                        ps = zp.tile([128, 8, 64], f32, tag="zp")
                        for j in range(nb):
                            gj = e0 // 128 + b0 + j
                            lj = gj * 128 - e0
                            nc.tensor.matmul(
                                ps[:, j, :],
                                lhsT=eat_c[:, lj:lj + 128],
                                rhs=W1cb[:], start=True, stop=True)
                        nc.vector._custom_dve(
                            RELU_ADD, out=zt[:, b0:b0 + nb, :],
                            in0=ps[:, 0:nb, :], in1=rc[:, b0:b0 + nb, :])
                    for j in range(ntc):
                        gj = e0 // 128 + j
                        for (pi, w) in pair_of.get(gj, []):
                            S_t = sbr.tile([128, 128], bf16, tag="St")
                            nc.vector.tensor_scalar(
                                out=S_t[:], in0=iotaF[:],
                                scalar1=lrowW[:, pi:pi + 1], scalar2=None,
                                op0=ALU.is_equal)
                            if w not in win_ps:
                                win_ps[w] = gp.tile([64, 128], f32, tag="gp", name=f"gps_{w}")
                            nc.tensor.matmul(
                                win_ps[w][:], lhsT=zt[:, j, :], rhs=S_t[:],
                                start=(pi == wf[w]), stop=(pi == wl[w]))
                            if pi == wl[w]:
                                dst = gT[:, w * 128:(w + 1) * 128]
                                if gname == "lo":
                                    nc.scalar.activation(out=dst, in_=win_ps[w][:],
                                                         func=AF.Copy)
                                else:
                                    nc.vector.tensor_tensor(out=dst, in0=dst,
                                                            in1=win_ps[w][:],
                                                            op=ALU.add)
                                del win_ps[w]
                assert not win_ps
            if not compute_z:
                nc.vector.tensor_scalar_min(out=dis_sb[:], in0=dis_sb[:],
                                            scalar1=1e4)
                nc.sync.dma_start(out=dis_out[t][:], in_=dis_sb[:])

        def node_pass():
            nbank = (NW + 7) // 8
            for b in range(nbank):
                nt = min(8, NW - b * 8)
                ps = xp.tile([128, 8, 64], f32, tag="xp")
                for j in range(nt):
                    m = b * 8 + j
                    nc.tensor.matmul(ps[:, j, :],
                                     lhsT=hTx[:, m * 128:(m + 1) * 128],
                                     rhs=WX[:], start=True, stop=False)
                    nc.tensor.matmul(ps[:, j, :],
                                     lhsT=gT[:, m * 128:(m + 1) * 128],
                                     rhs=W2d[:], start=False, stop=True)
                nc.scalar.activation(out=x_sb[:, b * 8:b * 8 + nt, :],
                                     in_=ps[:, 0:nt, :], func=AF.Copy)
                nc.vector.tensor_reduce(out=stat_mu[:, b * 8:b * 8 + nt],
                                        in_=x_sb[:, b * 8:b * 8 + nt, :],
                                        axis=mybir.AxisListType.X, op=ALU.add)
                x2 = sbr.tile([128, 8, 64], f32, tag="x2")
                nc.vector.tensor_tensor(out=x2[:, 0:nt, :],
                                        in0=x_sb[:, b * 8:b * 8 + nt, :],
                                        in1=x_sb[:, b * 8:b * 8 + nt, :],
                                        op=ALU.mult)
                nc.vector.tensor_reduce(out=stat_sq[:, b * 8:b * 8 + nt],
                                        in_=x2[:, 0:nt, :],
                                        axis=mybir.AxisListType.X, op=ALU.add)
            # mu = sum/64 ; var = sumsq/64 - mu^2 ; rstd = sqrt(1/(var+eps))
            nc.vector.tensor_scalar_mul(out=stat_mu[:], in0=stat_mu[:],
                                        scalar1=1.0 / 64)
            nc.vector.scalar_tensor_tensor(out=stat_rv[:], in0=stat_mu[:],
                                           scalar=-1.0, in1=stat_mu[:],
                                           op0=ALU.mult, op1=ALU.mult)
            nc.vector.scalar_tensor_tensor(out=stat_sq[:], in0=stat_sq[:],
                                           scalar=1.0 / 64, in1=stat_rv[:],
                                           op0=ALU.mult, op1=ALU.add)
            nc.vector.tensor_scalar_add(out=stat_sq[:], in0=stat_sq[:],
                                        scalar1=EPS)
            nc.vector.reciprocal(out=stat_rv[:], in_=stat_sq[:])
            nc.scalar.activation(out=stat_rv[:], in_=stat_rv[:], func=AF.Sqrt)
            nc.vector.scalar_tensor_tensor(out=stat_b[:], in0=stat_mu[:],
                                           scalar=-1.0, in1=stat_rv[:],
                                           op0=ALU.mult, op1=ALU.mult)
            for m in range(NW):
                xn = sbr.tile([128, 64], bf16, tag="xn")
                nc.vector.tensor_scalar(out=xn[:], in0=x_sb[:, m, :],
                                        scalar1=stat_rv[:, m:m + 1],
                                        scalar2=stat_b[:, m:m + 1],
                                        op0=ALU.mult, op1=ALU.add)
                tps = tp.tile([64, 128], bf16, tag="tp")
                nc.tensor.transpose(tps[:], xn[:], identB[:])
                sT = sbr.tile([64, 128], bf16, tag="sT")
                nc.scalar.activation(out=sT[:], in_=tps[:], func=AF.Silu,
                                     scale=lng[:], bias=lnb[:])
                yp = tp.tile([64, 128], f32, tag="tp")
                nc.tensor.matmul(yp[:], lhsT=Wd2[:], rhs=sT[:],
                                 start=True, stop=True)
                nc.scalar.activation(out=fT[:, m * 128:(m + 1) * 128], in_=yp[:],
                                     func=AF.Tanh, bias=bd2[:])
            nc.vector.scalar_tensor_tensor(out=hTx[0:64, 0:Nc],
                                           in0=fT[:, 0:Nc], scalar=STEP,
                                           in1=hTx[0:64, 0:Nc],
                                           op0=ALU.mult, op1=ALU.add)

        # ---------------- main loop ----------------
        for t in range(T):
            s = pp["per_t"][t]
            eaT_d = dmi[f"eaT{t}"]
            idxlo = sb.tile([128, max(s["NLOP"], 16) // 16], i16, tag="idxlo")
            nc.sync.dma_start(out=idxlo[:], in_=dmi[f"idxlo{t}"][:])
            idxhi = sb.tile([128, max(s["NHIP"], 16) // 16], i16, tag="idxhi")
            nc.sync.dma_start(out=idxhi[:], in_=dmi[f"idxhi{t}"][:])
            idxrow = sb.tile([128, s["NEP"] // 16], i16, tag="idxrow")
            nc.sync.dma_start(out=idxrow[:], in_=dmi[f"idxrow{t}"][:])
            lrowW = sb.tile([128, s["NPAIR"]], f32, tag="lrowW")
            nc.sync.dma_start(out=lrowW[:], in_=dmi[f"lrowW{t}"][:])
            nc.sync.dma_start(out=hTx[65:66, 0:Nc], in_=deg_d[t])

            if t > 0:
                nc.sync.dma_start(out=gT[0:64, 0:Nc], in_=hTseq_d[t])
                nc.vector.tensor_tensor(out=hTx[0:64, 0:Nc],
                                        in0=hTx[0:64, 0:Nc],
                                        in1=gT[0:64, 0:Nc], op=ALU.add)

            for _step in range(T_STEPS):
                par = (t * (T_STEPS + 1) + _step) % 2
                node_tables(write_h=False, par=par)
                allgather_tab(par)
                edge_pass(t, s, eaT_d, idxlo, idxhi, idxrow, lrowW,
                          compute_z=True, par=par)
                node_pass()

            par = (t * (T_STEPS + 1) + T_STEPS) % 2
            node_tables(write_h=True, par=par)
            allgather_tab(par)
            edge_pass(t, s, eaT_d, idxlo, idxhi, idxrow, lrowW,
                      compute_z=False, par=par)

        nc.sync.dma_start(out=hT_out[:], in_=hTx[0:64, 0:Nc])

    nc.compile()
    return nc


# --------------------------------------------------------------------------- #
# top-level
# --------------------------------------------------------------------------- #

def _run(h_seq, edge_attr, W1, b1, W2, b2, Wd1, bd1, ln_g, ln_b, Wd2, bd2,
         edge_index, split=32768):
    from concourse.bass_utils import run_bass_kernel_spmd

    h_seq = np.asarray(h_seq, np.float32)
    edge_attr = np.asarray(edge_attr, np.float32)
    edge_index = np.asarray(edge_index)
    W1 = np.asarray(W1, np.float32); W2 = np.asarray(W2, np.float32)
    Wd1 = np.asarray(Wd1, np.float32); Wd2_ = np.asarray(Wd2, np.float32)
    b1 = np.asarray(b1, np.float32); b2 = np.asarray(b2, np.float32)
    bd1 = np.asarray(bd1, np.float32); bd2_ = np.asarray(bd2, np.float32)
    ln_g = np.asarray(ln_g, np.float32); ln_b = np.asarray(ln_b, np.float32)

    T, N, D = h_seq.shape
    E = edge_index.shape[2]
    pp = _preprocess(h_seq, edge_attr, edge_index, split)
    Nc = pp["Nc"]

    W1a, W1b, W1c = W1[:D], W1[D:2 * D], W1[2 * D:]
    Wd1h, Wd1a = Wd1[:D], Wd1[D:]
    weights = {
        "WX": np.concatenate([Wd1h, bd1[None], (b2 @ Wd1a)[None]], 0
                             ).astype(np.float32),
        "W1ab": np.concatenate([W1a, W1b], 1).astype(np.float32),
        "W2d": (W2 @ Wd1a).astype(np.float32),
        "W1cb": np.concatenate([W1c, b1[None]], 0).astype(ml_dtypes.bfloat16),
        "Wd2": Wd2_.astype(ml_dtypes.bfloat16),
        "lng": np.ascontiguousarray(ln_g[:, None]),
        "lnb": np.ascontiguousarray(ln_b[:, None]),
        "bd2": np.ascontiguousarray(bd2_[:, None]),
    }

    nc = _build_program(pp)

    in_maps = []
    for k in range(N_CORES):
        m = dict(weights)
        m["hTseq"] = pp["hTseq"][k]
        m["degT"] = np.stack([pp["per_t"][t]["deg"][k][None] for t in range(T)])
        for t in range(T):
            s = pp["per_t"][t]
            m[f"eaT{t}"] = np.asarray(s["eaT"][k])
            m[f"idxlo{t}"] = s["idx_lo"][k]
            m[f"idxhi{t}"] = s["idx_hi"][k]
            m[f"idxrow{t}"] = s["idx_row"][k]
            m[f"lrowW{t}"] = np.asarray(s["lrowW"][k])
        in_maps.append(m)

    import os, time as _time
    trace = bool(int(os.environ.get("GNN_TRACE", "0")))
    if trace:
        try:
            from antenv.axon_hooks import get_axon_ntff_profile_hook  # noqa: F401
        except ImportError:
            trace = False
    _t0 = _time.time()
    res = run_bass_kernel_spmd(nc, in_maps, list(range(N_CORES)), trace=trace)
    global LAST_EXEC_NS, LAST_RES, LAST_RUN_WALL
    LAST_RUN_WALL = _time.time() - _t0
    LAST_EXEC_NS = res.exec_time_ns
    LAST_RES = res

    h_final = np.zeros((N, D), np.float32)
    dis = np.zeros((T, E), np.float32)
    for k in range(N_CORES):
        r = res.results[k]
        h_final[k * Nc:(k + 1) * Nc] = r["hT_out"].T
        for t in range(T):
            s = pp["per_t"][t]
            dd = r[f"dis{t}"]
            ids, pos = s["eid_pos"][k]
            dis[t, ids] = dd[pos % 128, pos // 128]
    return h_final, dis


def kernel(**inputs):
    return _run(**inputs)


# revision 11
# speedup vs baseline: 1.3064x; 1.3064x over previous
"""Trainium2 Bass kernel for nn_ContinuousSheafTransport (GNN message-passing ODE).

Self-contained: takes FULL inputs, shards across 8 NeuronCores internally,
returns FULL outputs (h_final [N,D], disagreements [T,E]).

Sharding: nodes split 8 ways; each edge lives on the core owning its
destination row; edges sorted by local row so segment_sum is core-local.
Per Euler step each core builds node-level tables hA=h@W1[:D], hB=h@W1[D:2D]
(hB all-gathered); per-edge work reduces to two 256B dma_gathers, a relu, and
a one-hot segment-matmul:
    z_e = relu(hA[row_e] + hB[col_e] + ea_e@W1c + b1)
    x   = h@Wd1h + segsum(z)@(W2@Wd1a) + (deg*b2)@Wd1a + bd1
    h  += 0.5*tanh(silu(LN(x))@Wd2 + bd2)
The node pipeline runs transposed (features on partitions) so LN gamma/beta,
bd2 and tanh are per-partition scalar-engine ops.
"""
import numpy as np
import ml_dtypes

T_STEPS = 2
LAST_EXEC_NS = None
LAST_RES = None
LAST_RUN_WALL = None
STEP = 0.5
EPS = 1e-5
N_CORES = 8
CH = 1024        # gather chunk size in edges (dma_gather crashes above ~1024 idxs)


# --------------------------------------------------------------------------- #
# host-side preprocessing
# --------------------------------------------------------------------------- #

def _wrap_idx(idx):
    """dma_gather int16 idx layout [128, n/16]: element i at [i%16, i//16]."""
    n = len(idx)
    assert n % 16 == 0
    blk = idx.reshape(n // 16, 16).T.astype(np.int16)
    return np.tile(blk, (8, 1))


def _preprocess(h_seq, edge_attr, edge_index, split):
    T, N, D = h_seq.shape
    NC = N_CORES
    assert N % NC == 0
    Nc = N // NC
    NW = (Nc + 127) // 128

    pp = {"T": T, "N": N, "D": D, "E": edge_index.shape[2], "Nc": Nc, "NW": NW,
          "split": split}
    per_t = []
    for t in range(T):
        row = np.asarray(edge_index[t, 0], np.int64)
        col = np.asarray(edge_index[t, 1], np.int64)
        ea = np.asarray(edge_attr[t], np.float32)

        cores, nlo_max, nhi_max = [], 0, 0
        for k in range(NC):
            eids = np.nonzero((row >= k * Nc) & (row < (k + 1) * Nc))[0]
            lr = row[eids] - k * Nc
            is_lo = col[eids] < split
            lo = eids[is_lo][np.argsort(lr[is_lo], kind="stable")]
            hi = eids[~is_lo][np.argsort(lr[~is_lo], kind="stable")]
            cores.append((lo, hi))
            nlo_max, nhi_max = max(nlo_max, len(lo)), max(nhi_max, len(hi))
        NLOP = ((nlo_max + 127) // 128) * 128
        NHIP = ((nhi_max + 127) // 128) * 128
        NEP = NLOP + NHIP
        NTIL = NEP // 128

        eaT = np.zeros((NC, 17, NEP), ml_dtypes.bfloat16)
        col_lo = np.zeros((NC, max(NLOP, 16)), np.int64)
        col_hi = np.zeros((NC, max(NHIP, 16)), np.int64)
        row_l = np.zeros((NC, NEP), np.int64)
        lvalid = np.zeros((NC, NEP), bool)
        eid_pos = []
        for k in range(NC):
            lo, hi = cores[k]
            pos = np.concatenate([np.arange(len(lo)), NLOP + np.arange(len(hi))])
            ids = np.concatenate([lo, hi])
            eid_pos.append((ids, pos))
            eaT[k, :16, pos] = ea[ids].astype(ml_dtypes.bfloat16)
            eaT[k, 16, pos] = np.float32(1.0)
            col_lo[k, :len(lo)] = col[lo]
            col_hi[k, :len(hi)] = col[hi] - split
            row_l[k, pos] = row[ids] - k * Nc
            lvalid[k, pos] = True

        # union-over-cores (tile, window) schedule, per group
        def build_pairs(j0, j1, pbase):
            pair_set = {}
            for k in range(NC):
                for j in range(j0, j1):
                    sl = slice(j * 128, (j + 1) * 128)
                    v = lvalid[k, sl]
                    if not v.any():
                        continue
                    for w in np.unique(row_l[k, sl][v] // 128):
                        pair_set.setdefault(j, set()).add(int(w))
            pairs = []
            for j in range(j0, j1):
                for w in sorted(pair_set.get(j, ())):
                    pairs.append((j, int(w), pbase + len(pairs)))
            wf, wl = {}, {}
            for (j, w, pi) in pairs:
                wf.setdefault(w, pi)
                wl[w] = pi
            return pairs, wf, wl

        pairs_lo, wf_lo, wl_lo = build_pairs(0, NLOP // 128, 0)
        pairs_hi, wf_hi, wl_hi = build_pairs(NLOP // 128, NTIL, len(pairs_lo))
        NPAIR = max(len(pairs_lo) + len(pairs_hi), 1)
        lrowW = np.full((NC, 128, NPAIR), -1.0, np.float32)
        for (j, w, pi) in pairs_lo + pairs_hi:
            sl = slice(j * 128, (j + 1) * 128)
            for k in range(NC):
                lv = row_l[k, sl] - 128 * w
                ok = lvalid[k, sl] & (lv >= 0) & (lv < 128)
                lrowW[k, ok, pi] = lv[ok]

        deg = np.zeros((NC, Nc), np.float32)
        for k in range(NC):
            ids, _ = eid_pos[k]
            np.add.at(deg[k], row[ids] - k * Nc, 1.0)

        per_t.append({
            "NLOP": NLOP, "NHIP": NHIP, "NEP": NEP, "NTIL": NTIL,
            "pairs_lo": pairs_lo, "pairs_hi": pairs_hi,
            "wf_lo": wf_lo, "wl_lo": wl_lo, "wf_hi": wf_hi, "wl_hi": wl_hi,
            "NPAIR": NPAIR,
            "eaT": eaT,
            "idx_lo": np.stack([_wrap_idx(col_lo[k]) for k in range(NC)]),
            "idx_hi": np.stack([_wrap_idx(col_hi[k]) for k in range(NC)]),
            "idx_row": np.stack([_wrap_idx(row_l[k]) for k in range(NC)]),
            "lrowW": lrowW,
            "deg": deg, "eid_pos": eid_pos,
        })
    pp["per_t"] = per_t
    pp["hTseq"] = np.ascontiguousarray(
        h_seq.reshape(T, NC, Nc, D).transpose(1, 0, 3, 2)).astype(np.float32)
    return pp


# --------------------------------------------------------------------------- #
# device program
# --------------------------------------------------------------------------- #

def _build_program(pp):
    import concourse.bacc as bacc
    import concourse.mybir as mybir
    import concourse.tile as tile
    from contextlib import ExitStack
    from concourse.dve_spec import Spec, Src0, Src1, relu as drelu, sq as dsq, \
        lower as dve_lower
    from concourse.dve_uop import DveOpSpec
    from concourse import dve_ops as dvo

    f32, bf16, i16, i32 = (mybir.dt.float32, mybir.dt.bfloat16,
                           mybir.dt.int16, mybir.dt.int32)
    AF = mybir.ActivationFunctionType
    ALU = mybir.AluOpType

    def make_op(name, spec):
        for o in dvo.OPS:
            if o.name == name:
                return o
        shas = {}
        for ver in ("v3", "v4"):
            tmp = DveOpSpec(name=name, opcode=0, uops=dve_lower(spec, ver=ver),
                            rd1_en=True)
            shas[ver] = tmp.sha(ver)
        op = dvo.DveOp(name, spec, subdim=False, uops_sha=shas)
        dvo.OPS.append(op)
        dvo.CUSTOM_DVE_SPECS[name] = spec
        dvo._SUB_OPCODE_FOR_NAME[name] = dvo._CUSTOM_DVE_ROW_BASE + len(dvo.OPS) - 1
        assert dvo._SUB_OPCODE_FOR_NAME[name] < 0x20
        return op

    RELU_ADD = make_op("GNN_RELU_ADD", Spec(
        body=drelu(Src0 + Src1),
        reference=lambda in0, in1, s0, s1, imm2:
            np.maximum(np.nan_to_num(in0.astype(np.float32) + in1,
                                     nan=0.0, posinf=np.inf, neginf=-np.inf), 0)))
    SQDIFF = make_op("GNN_SQDIFF", Spec(
        body=dsq(Src0 - Src1),
        reference=lambda in0, in1, s0, s1, imm2:
            np.square(in0.astype(np.float32) - in1)))

    T, Nc, NW = pp["T"], pp["Nc"], pp["NW"]
    N, split = pp["N"], pp["split"]
    NcP = NW * 128

    nc = bacc.Bacc("TRN2", target_bir_lowering=False, debug=False,
                   num_devices=N_CORES)

    dmi = {}

    def din(name, shape, dt):
        dmi[name] = nc.dram_tensor(name, list(shape), dt, kind="ExternalInput")
        return dmi[name]

    hTseq_d = din("hTseq", [T, 64, Nc], f32)
    WX_d = din("WX", [66, 64], f32)
    W1ab_d = din("W1ab", [64, 128], f32)
    W2d_d = din("W2d", [64, 64], f32)
    W1cb_d = din("W1cb", [17, 64], bf16)
    Wd2_d = din("Wd2", [64, 64], bf16)
    lng_d = din("lng", [64, 1], f32)
    lnb_d = din("lnb", [64, 1], f32)
    bd2_d = din("bd2", [64, 1], f32)
    deg_d = din("degT", [T, 1, Nc], f32)
    for t in range(T):
        s = pp["per_t"][t]
        din(f"eaT{t}", [17, s["NEP"]], bf16)
        din(f"idxlo{t}", [128, max(s["NLOP"], 16) // 16], i16)
        din(f"idxhi{t}", [128, max(s["NHIP"], 16) // 16], i16)
        din(f"idxrow{t}", [128, s["NEP"] // 16], i16)
        din(f"lrowW{t}", [128, s["NPAIR"]], f32)

    hT_out = nc.dram_tensor("hT_out", [64, Nc], f32, kind="ExternalOutput")
    dis_out = {t: nc.dram_tensor(f"dis{t}", [128, pp["per_t"][t]["NTIL"]], f32,
                                 kind="ExternalOutput") for t in range(T)}

    hA_loc2 = [nc.dram_tensor(f"hA_loc{p}", [NcP, 64], f32) for p in range(2)]
    tab_in2 = [nc.dram_tensor(f"tab_in{p}", [Nc, 64], f32) for p in range(2)]
    tab_full2 = [nc.dram_tensor(f"tab_full{p}", [N, 64], f32, addr_space="Shared")
                 for p in range(2)]

    with tile.TileContext(nc) as tc, ExitStack() as ctx:
        sb = ctx.enter_context(tc.tile_pool(name="sb", bufs=1))
        sbr = ctx.enter_context(tc.tile_pool(name="sbr", bufs=3))
        zp = ctx.enter_context(tc.tile_pool(name="zp", bufs=2, space="PSUM"))
        gp = ctx.enter_context(tc.tile_pool(name="gp", bufs=2, space="PSUM"))
        xp = ctx.enter_context(tc.tile_pool(name="xp", bufs=2, space="PSUM"))
        tp = ctx.enter_context(tc.tile_pool(name="tp", bufs=2, space="PSUM"))

        # ---------------- residents ----------------
        hTx = sb.tile([66, NcP], f32)
        nc.vector.memset(hTx[:], 0.0)
        WX = sb.tile([66, 64], f32); nc.sync.dma_start(out=WX[:], in_=WX_d[:])
        W1ab = sb.tile([64, 128], f32); nc.sync.dma_start(out=W1ab[:], in_=W1ab_d[:])
        W2d = sb.tile([64, 64], f32); nc.sync.dma_start(out=W2d[:], in_=W2d_d[:])
        W1cb = sb.tile([17, 64], bf16); nc.sync.dma_start(out=W1cb[:], in_=W1cb_d[:])
        Wd2 = sb.tile([64, 64], bf16); nc.sync.dma_start(out=Wd2[:], in_=Wd2_d[:])
        lng = sb.tile([64, 1], f32); nc.sync.dma_start(out=lng[:], in_=lng_d[:])
        lnb = sb.tile([64, 1], f32); nc.sync.dma_start(out=lnb[:], in_=lnb_d[:])
        bd2 = sb.tile([64, 1], f32); nc.sync.dma_start(out=bd2[:], in_=bd2_d[:])

        nc.vector.memset(hTx[64:65, :], 1.0)      # ones row

        iota_i = sb.tile([128, 128], i32)
        nc.gpsimd.iota(iota_i[:], pattern=[[1, 128]], base=0, channel_multiplier=0)
        iotaF = sb.tile([128, 128], f32)
        nc.vector.tensor_copy(out=iotaF[:], in_=iota_i[:])
        iotaP_i = sb.tile([128, 1], i32)
        nc.gpsimd.iota(iotaP_i[:], pattern=[[0, 1]], base=0, channel_multiplier=1)
        iotaP = sb.tile([128, 1], f32)
        nc.vector.tensor_copy(out=iotaP[:], in_=iotaP_i[:])
        identB = sb.tile([128, 128], bf16)
        nc.vector.tensor_scalar(out=identB[:], in0=iotaF[:], scalar1=iotaP[:],
                                scalar2=None, op0=ALU.is_equal)
        identF = sb.tile([128, 128], f32)
        nc.vector.tensor_copy(out=identF[:], in_=identB[:])
        KMAX = 32
        iotaFK_i = sb.tile([128, KMAX, 128], i32)
        nc.gpsimd.iota(iotaFK_i[:], pattern=[[0, KMAX], [1, 128]], base=0,
                       channel_multiplier=0)
        iotaFK = sb.tile([128, KMAX, 128], f32)
        nc.vector.tensor_copy(out=iotaFK[:], in_=iotaFK_i[:])

        # h init: h = 2*h_seq[0]
        nc.sync.dma_start(out=hTx[0:64, 0:Nc], in_=hTseq_d[0])
        nc.vector.tensor_scalar_mul(out=hTx[0:64, 0:Nc], in0=hTx[0:64, 0:Nc],
                                    scalar1=2.0)

        gT = sb.tile([64, NcP], f32)
        fT = sb.tile([64, NcP], bf16)
        x_sb = sb.tile([128, NW, 64], f32)
        stat_mu = sb.tile([128, NW], f32)
        stat_sq = sb.tile([128, NW], f32)
        stat_rv = sb.tile([128, NW], f32)
        stat_b = sb.tile([128, NW], f32)

        # ---------------- helpers ----------------
        def node_tables(write_h, par):
            hA_loc, tab_in = hA_loc2[par], tab_in2[par]
            for m in range(NW):
                lo = min(128, Nc - m * 128)
                if lo <= 0:
                    continue
                ps = tp.tile([128, 128], f32, tag="tp")
                st = sbr.tile([128, 128], f32, tag="tabs")
                if write_h:
                    nc.tensor.transpose(ps[:, 0:64],
                                        hTx[0:64, m * 128:(m + 1) * 128],
                                        identF[0:64, 0:64])
                    nc.scalar.activation(out=st[:, 0:64], in_=ps[:, 0:64],
                                         func=AF.Copy)
                    nc.sync.dma_start(out=tab_in[m * 128:m * 128 + lo, :],
                                      in_=st[0:lo, 0:64])
                    nc.sync.dma_start(out=hA_loc[m * 128:m * 128 + lo, :],
                                      in_=st[0:lo, 0:64])
                else:
                    nc.tensor.matmul(ps[:], lhsT=hTx[0:64, m * 128:(m + 1) * 128],
                                     rhs=W1ab[:], start=True, stop=True)
                    nc.scalar.activation(out=st[:], in_=ps[:], func=AF.Copy)
                    nc.sync.dma_start(out=hA_loc[m * 128:m * 128 + lo, :],
                                      in_=st[0:lo, 0:64])
                    nc.sync.dma_start(out=tab_in[m * 128:m * 128 + lo, :],
                                      in_=st[0:lo, 64:128])

        def allgather_tab(par):
            nc.gpsimd.collective_compute(
                "AllGather", ALU.bypass,
                replica_groups=[list(range(N_CORES))],
                ins=[tab_in2[par][:]], outs=[tab_full2[par][:]])

        def edge_pass(t, s, eaT_d, idxlo, idxhi, idxrow, lrowW, compute_z, par):
            hA_loc, tab_full = hA_loc2[par], tab_full2[par]
            NLOP, NHIP, NEP, NTIL = s["NLOP"], s["NHIP"], s["NEP"], s["NTIL"]
            groups = []
            if NLOP:
                groups.append((0, NLOP, idxlo, tab_full[0:min(split, N), :],
                               s["pairs_lo"], s["wf_lo"], s["wl_lo"], "lo"))
            if NHIP:
                groups.append((NLOP, NHIP, idxhi, tab_full[split:N, :],
                               s["pairs_hi"], s["wf_hi"], s["wl_hi"], "hi"))

            if compute_z:
                nc.vector.memset(gT[:], 0.0)
            dis_sb = None
            if not compute_z:
                dis_sb = sb.tile([128, NTIL], f32, tag="dis_sb")

            for (g0, gcnt, idx_t, src, pairs, wf, wl, gname) in groups:
                pair_of = {}
                for (j, w, pi) in pairs:
                    pair_of.setdefault(j, []).append((pi, w))
                win_ps = {}
                nch = (gcnt + CH - 1) // CH
                LOOK = 3
                rg_bufs = {}

                def emit_rowgather(c):
                    e0 = g0 + c * CH
                    ecnt = min(CH, g0 + gcnt - e0)
                    rgb = sbr.tile([128, CH // 128, 64], f32, tag="rg",
                                   name=f"rg_{c}", bufs=LOOK + 2)
                    nc.gpsimd.dma_gather(
                        out_ap=rgb[:, 0:ecnt // 128, :], in_ap=hA_loc[:, :],
                        idxs_ap=idxrow[:, e0 // 16:(e0 + ecnt) // 16],
                        num_idxs=ecnt, num_idxs_reg=ecnt, elem_size=64)
                    rg_bufs[c] = rgb

                for c in range(min(LOOK, nch)):
                    emit_rowgather(c)
                for c in range(nch):
                    e0 = g0 + c * CH
                    ecnt = min(CH, g0 + gcnt - e0)
                    ntc = ecnt // 128
                    if c + LOOK < nch:
                        emit_rowgather(c + LOOK)
                    rg = rg_bufs.pop(c)
                    cg = sbr.tile([128, CH // 128, 64], f32, tag="cg")
                    nc.gpsimd.dma_gather(
                        out_ap=cg[:, 0:ntc, :], in_ap=src,
                        idxs_ap=idx_t[:, (e0 - g0) // 16:(e0 - g0 + ecnt) // 16],
                        num_idxs=ecnt, num_idxs_reg=ecnt, elem_size=64)

                    if not compute_z:
                        d2 = sbr.tile([128, CH // 128, 64], f32, tag="d2")
                        nc.vector._custom_dve(SQDIFF, out=d2[:, 0:ntc, :],
                                              in0=rg[:, 0:ntc, :],
                                              in1=cg[:, 0:ntc, :])
                        nc.vector.tensor_reduce(
                            out=dis_sb[:, e0 // 128:e0 // 128 + ntc],
                            in_=d2[:, 0:ntc, :], axis=mybir.AxisListType.X,
                            op=ALU.add)
                        continue

                    eat_c = sbr.tile([17, CH], bf16, tag="eat_c")
                    nc.sync.dma_start(out=eat_c[:, 0:ecnt],
                                      in_=eaT_d[:, e0:e0 + ecnt])
                    rc = sbr.tile([128, CH // 128, 64], bf16, tag="rc")
                    nc.vector.tensor_tensor(out=rc[:, 0:ntc, :],
                                            in0=rg[:, 0:ntc, :],
                                            in1=cg[:, 0:ntc, :], op=ALU.add)
                    zt = sbr.tile([128, CH // 128, 64], bf16, tag="zt")
                    for b0 in range(0, ntc, 8):
                        nb = min(8, ntc - b0)
                        ps = zp.tile([128, 8, 64], f32, tag="zp")
                        for j in range(nb):
                            gj = e0 // 128 + b0 + j
                            lj = gj * 128 - e0
                            nc.tensor.matmul(
                                ps[:, j, :],
                                lhsT=eat_c[:, lj:lj + 128],
                                rhs=W1cb[:], start=True, stop=True)
                        nc.vector._custom_dve(
                            RELU_ADD, out=zt[:, b0:b0 + nb, :],
                            in0=ps[:, 0:nb, :], in1=rc[:, b0:b0 + nb, :])
                    cpairs = []            # (j, pi, w) for this chunk, pi ascending
                    for j in range(ntc):
                        for (pi, w) in pair_of.get(e0 // 128 + j, []):
                            cpairs.append((j, pi, w))
                    if cpairs:
                        p0 = cpairs[0][1]
                        npc = cpairs[-1][1] - p0 + 1
                        S_all = sbr.tile([128, max(npc, 1), 128], bf16, tag="St",
                                         name=f"St_{gname}_{c}")
                        for q0 in range(0, npc, 32):
                            qn = min(32, npc - q0)
                            lwb = lrowW[:, p0 + q0:p0 + q0 + qn, None]                                 .broadcast_to((128, qn, 128))
                            nc.vector.tensor_tensor(
                                out=S_all[:, q0:q0 + qn, :],
                                in0=iotaFK[:, 0:qn, :], in1=lwb,
                                op=ALU.is_equal)
                    for (j, pi, w) in cpairs:
                        if w not in win_ps:
                            win_ps[w] = gp.tile([64, 128], f32, tag="gp", name=f"gps_{w}")
                        nc.tensor.matmul(
                            win_ps[w][:], lhsT=zt[:, j, :],
                            rhs=S_all[:, pi - p0, :],
                            start=(pi == wf[w]), stop=(pi == wl[w]))
                        if pi == wl[w]:
                            dst = gT[:, w * 128:(w + 1) * 128]
                            if gname == "lo":
                                nc.scalar.activation(out=dst, in_=win_ps[w][:],
                                                     func=AF.Copy)
                            else:
                                nc.vector.tensor_tensor(out=dst, in0=dst,
                                                        in1=win_ps[w][:],
                                                        op=ALU.add)
                            del win_ps[w]
                assert not win_ps
            if not compute_z:
                nc.vector.tensor_scalar_min(out=dis_sb[:], in0=dis_sb[:],
                                            scalar1=1e4)
                nc.sync.dma_start(out=dis_out[t][:], in_=dis_sb[:])

        def node_pass():
            nbank = (NW + 7) // 8
            for b in range(nbank):
                nt = min(8, NW - b * 8)
                ps = xp.tile([128, 8, 64], f32, tag="xp")
                for j in range(nt):
                    m = b * 8 + j
                    nc.tensor.matmul(ps[:, j, :],
                                     lhsT=hTx[:, m * 128:(m + 1) * 128],
                                     rhs=WX[:], start=True, stop=False)
                    nc.tensor.matmul(ps[:, j, :],
                                     lhsT=gT[:, m * 128:(m + 1) * 128],
                                     rhs=W2d[:], start=False, stop=True)
                nc.scalar.activation(out=x_sb[:, b * 8:b * 8 + nt, :],
                                     in_=ps[:, 0:nt, :], func=AF.Copy)
                nc.vector.tensor_reduce(out=stat_mu[:, b * 8:b * 8 + nt],
                                        in_=x_sb[:, b * 8:b * 8 + nt, :],
                                        axis=mybir.AxisListType.X, op=ALU.add)
                x2 = sbr.tile([128, 8, 64], f32, tag="x2")
                nc.vector.tensor_tensor(out=x2[:, 0:nt, :],
                                        in0=x_sb[:, b * 8:b * 8 + nt, :],
                                        in1=x_sb[:, b * 8:b * 8 + nt, :],
                                        op=ALU.mult)
                nc.vector.tensor_reduce(out=stat_sq[:, b * 8:b * 8 + nt],
                                        in_=x2[:, 0:nt, :],
                                        axis=mybir.AxisListType.X, op=ALU.add)
            # mu = sum/64 ; var = sumsq/64 - mu^2 ; rstd = sqrt(1/(var+eps))
            nc.vector.tensor_scalar_mul(out=stat_mu[:], in0=stat_mu[:],
                                        scalar1=1.0 / 64)
            nc.vector.scalar_tensor_tensor(out=stat_rv[:], in0=stat_mu[:],
                                           scalar=-1.0, in1=stat_mu[:],
                                           op0=ALU.mult, op1=ALU.mult)
            nc.vector.scalar_tensor_tensor(out=stat_sq[:], in0=stat_sq[:],
                                           scalar=1.0 / 64, in1=stat_rv[:],
                                           op0=ALU.mult, op1=ALU.add)
            nc.vector.tensor_scalar_add(out=stat_sq[:], in0=stat_sq[:],
                                        scalar1=EPS)
            nc.vector.reciprocal(out=stat_rv[:], in_=stat_sq[:])
            nc.scalar.activation(out=stat_rv[:], in_=stat_rv[:], func=AF.Sqrt)
            nc.vector.scalar_tensor_tensor(out=stat_b[:], in0=stat_mu[:],
                                           scalar=-1.0, in1=stat_rv[:],
                                           op0=ALU.mult, op1=ALU.mult)
            for m in range(NW):
                xn = sbr.tile([128, 64], bf16, tag="xn")
                nc.vector.tensor_scalar(out=xn[:], in0=x_sb[:, m, :],
                                        scalar1=stat_rv[:, m:m + 1],
                                        scalar2=stat_b[:, m:m + 1],
                                        op0=ALU.mult, op1=ALU.add)
                tps = tp.tile([64, 128], bf16, tag="tp")
                nc.tensor.transpose(tps[:], xn[:], identB[:])
                sT = sbr.tile([64, 128], bf16, tag="sT")
                nc.scalar.activation(out=sT[:], in_=tps[:], func=AF.Silu,
                                     scale=lng[:], bias=lnb[:])
                yp = tp.tile([64, 128], f32, tag="tp")
                nc.tensor.matmul(yp[:], lhsT=Wd2[:], rhs=sT[:],
                                 start=True, stop=True)
                nc.scalar.activation(out=fT[:, m * 128:(m + 1) * 128], in_=yp[:],
                                     func=AF.Tanh, bias=bd2[:])
            nc.vector.scalar_tensor_tensor(out=hTx[0:64, 0:Nc],
                                           in0=fT[:, 0:Nc], scalar=STEP,
                                           in1=hTx[0:64, 0:Nc],
                                           op0=ALU.mult, op1=ALU.add)

        # ---------------- main loop ----------------
        for t in range(T):
            s = pp["per_t"][t]
            eaT_d = dmi[f"eaT{t}"]
            idxlo = sb.tile([128, max(s["NLOP"], 16) // 16], i16, tag="idxlo")
            nc.sync.dma_start(out=idxlo[:], in_=dmi[f"idxlo{t}"][:])
            idxhi = sb.tile([128, max(s["NHIP"], 16) // 16], i16, tag="idxhi")
            nc.sync.dma_start(out=idxhi[:], in_=dmi[f"idxhi{t}"][:])
            idxrow = sb.tile([128, s["NEP"] // 16], i16, tag="idxrow")
            nc.sync.dma_start(out=idxrow[:], in_=dmi[f"idxrow{t}"][:])
            lrowW = sb.tile([128, s["NPAIR"]], f32, tag="lrowW")
            nc.sync.dma_start(out=lrowW[:], in_=dmi[f"lrowW{t}"][:])
            nc.sync.dma_start(out=hTx[65:66, 0:Nc], in_=deg_d[t])

            if t > 0:
                nc.sync.dma_start(out=gT[0:64, 0:Nc], in_=hTseq_d[t])
                nc.vector.tensor_tensor(out=hTx[0:64, 0:Nc],
                                        in0=hTx[0:64, 0:Nc],
                                        in1=gT[0:64, 0:Nc], op=ALU.add)

            for _step in range(T_STEPS):
                par = (t * (T_STEPS + 1) + _step) % 2
                node_tables(write_h=False, par=par)
                allgather_tab(par)
                edge_pass(t, s, eaT_d, idxlo, idxhi, idxrow, lrowW,
                          compute_z=True, par=par)
                node_pass()

            par = (t * (T_STEPS + 1) + T_STEPS) % 2
            node_tables(write_h=True, par=par)
            allgather_tab(par)
            edge_pass(t, s, eaT_d, idxlo, idxhi, idxrow, lrowW,
                      compute_z=False, par=par)

        nc.sync.dma_start(out=hT_out[:], in_=hTx[0:64, 0:Nc])

    nc.compile()
    return nc


# --------------------------------------------------------------------------- #
# top-level
# --------------------------------------------------------------------------- #

def _run(h_seq, edge_attr, W1, b1, W2, b2, Wd1, bd1, ln_g, ln_b, Wd2, bd2,
         edge_index, split=32768):
    from concourse.bass_utils import run_bass_kernel_spmd

    h_seq = np.asarray(h_seq, np.float32)
    edge_attr = np.asarray(edge_attr, np.float32)
    edge_index = np.asarray(edge_index)
    W1 = np.asarray(W1, np.float32); W2 = np.asarray(W2, np.float32)
    Wd1 = np.asarray(Wd1, np.float32); Wd2_ = np.asarray(Wd2, np.float32)
    b1 = np.asarray(b1, np.float32); b2 = np.asarray(b2, np.float32)
    bd1 = np.asarray(bd1, np.float32); bd2_ = np.asarray(bd2, np.float32)
    ln_g = np.asarray(ln_g, np.float32); ln_b = np.asarray(ln_b, np.float32)

    T, N, D = h_seq.shape
    E = edge_index.shape[2]
    pp = _preprocess(h_seq, edge_attr, edge_index, split)
    Nc = pp["Nc"]

    W1a, W1b, W1c = W1[:D], W1[D:2 * D], W1[2 * D:]
    Wd1h, Wd1a = Wd1[:D], Wd1[D:]
    weights = {
        "WX": np.concatenate([Wd1h, bd1[None], (b2 @ Wd1a)[None]], 0
                             ).astype(np.float32),
        "W1ab": np.concatenate([W1a, W1b], 1).astype(np.float32),
        "W2d": (W2 @ Wd1a).astype(np.float32),
        "W1cb": np.concatenate([W1c, b1[None]], 0).astype(ml_dtypes.bfloat16),
        "Wd2": Wd2_.astype(ml_dtypes.bfloat16),
        "lng": np.ascontiguousarray(ln_g[:, None]),
        "lnb": np.ascontiguousarray(ln_b[:, None]),
        "bd2": np.ascontiguousarray(bd2_[:, None]),
    }

    nc = _build_program(pp)

    in_maps = []
    for k in range(N_CORES):
        m = dict(weights)
        m["hTseq"] = pp["hTseq"][k]
        m["degT"] = np.stack([pp["per_t"][t]["deg"][k][None] for t in range(T)])
        for t in range(T):
            s = pp["per_t"][t]
            m[f"eaT{t}"] = np.asarray(s["eaT"][k])
            m[f"idxlo{t}"] = s["idx_lo"][k]
            m[f"idxhi{t}"] = s["idx_hi"][k]
            m[f"idxrow{t}"] = s["idx_row"][k]
            m[f"lrowW{t}"] = np.asarray(s["lrowW"][k])
        in_maps.append(m)

    import os, time as _time
    trace = bool(int(os.environ.get("GNN_TRACE", "0")))
    if trace:
        try:
            from antenv.axon_hooks import get_axon_ntff_profile_hook  # noqa: F401
        except ImportError:
            trace = False
    _t0 = _time.time()
    res = run_bass_kernel_spmd(nc, in_maps, list(range(N_CORES)), trace=trace)
    global LAST_EXEC_NS, LAST_RES, LAST_RUN_WALL
    LAST_RUN_WALL = _time.time() - _t0
    LAST_EXEC_NS = res.exec_time_ns
    LAST_RES = res

    h_final = np.zeros((N, D), np.float32)
    dis = np.zeros((T, E), np.float32)
    for k in range(N_CORES):
        r = res.results[k]
        h_final[k * Nc:(k + 1) * Nc] = r["hT_out"].T
        for t in range(T):
            s = pp["per_t"][t]
            dd = r[f"dis{t}"]
            ids, pos = s["eid_pos"][k]
            dis[t, ids] = dd[pos % 128, pos // 128]
    return h_final, dis


def kernel(**inputs):
    return _run(**inputs)
